# revision 16
# baseline (speedup 1.0000x reference)
"""Trainium2 Bass kernel: CQT (constant-Q transform) of 2^23 audio samples.

Reference math (jax):
    frames[f, n] = x[f*HOP + n]                  HOP=512, fftLen=2048
    four_r = frames @ wcos.T ; four_i = frames @ wsin.T
    cqt_r  = kr @ four_r - ki @ four_i
    cqt_i  = kr @ four_i + ki @ four_r
    out    = sqrt(cqt_r**2 + cqt_i**2)           # [1, 84, n_frames]

Folded on the host (exact algebra, tiny matrices):
    A = kr@wcos - ki@wsin,  B = kr@wsin + ki@wcos      (each [84, 2048])
    out = sqrt((A @ frames.T)**2 + (B @ frames.T)**2)

Device strategy (8-way shard along the frame axis; kernels replicated):
  - 2048 frames per core; its x-shard is cast to bf16 and laid out on the
    host as xt[p, c] = x[c*128 + p], so frames.T for contraction chunk kc
    of frame f is the strided SBUF column view 4*f + kc — the overlapped
    frame matrix is never materialized.  A.T/B.T ride the same DRAM tensor.
  - one plain partition-major DMA in; 4 frame blocks x 16 K-chunks x {A,B}
    matmuls accumulate in PSUM; a^2+b^2 on VectorE; one DMA out.
  - sqrt is taken on the host (monotone + exact in fp32); the device returns
    squared magnitudes.  This keeps the kernel to 4 sync procs (PE, DVE, one
    HWDGE lane, one SWDGE lane) — the walrus tail-drain supports at most 4
    sem waits and any DMA at most 1.
"""

import sys

if "/opt/trn_rl_repo" not in sys.path:
    sys.path.insert(0, "/opt/trn_rl_repo")

import numpy as np
import ml_dtypes

HOP = 512
FFTLEN = 2048
N_BINS = 84
T_SAMPLES = 8388608
N_FRAMES = (T_SAMPLES - FFTLEN) // HOP + 1  # 16381
N_CORES = 8
F_PER_CORE = 2048                 # frames computed per core (3 junk at the end)
X_COLS = 8256                     # xt columns for x; 8256*128 = 1056768 samples
SHARD_LEN = X_COLS * 128
CORE_STRIDE = F_PER_CORE * HOP    # 1048576 samples between shard starts
N_KC = FFTLEN // 128              # 16 contraction chunks
N_FB = F_PER_CORE // 512          # 4 frame blocks of 512 frames
AB_COLS = N_KC * 2 * N_BINS       # 2688 columns holding A.T/B.T chunks
EXT_COLS = X_COLS + AB_COLS       # 10944

_PROGRAM = None


def _split_multi_waits(nc, mybir, max_waits=1):
    """This walrus build encodes at most one sem wait per instruction; move
    extra waits onto injected same-engine NoOps right before the instruction."""
    ctr = 0
    for f in nc.m.functions:
        for blk in f.blocks:
            il = list(blk.instructions)
            new = []
            changed = False
            for inst in il:
                si = getattr(inst, "sync_info", None)
                if si is not None and len(si.on_wait) > max_waits:
                    waits = list(si.on_wait)
                    for w in waits[:-max_waits]:
                        nop = mybir.InstNoOp(name=f"I-waitfix-{ctr}", ins=[], outs=[])
                        ctr += 1
                        nop.engine = inst.engine
                        nop.sync_info = mybir.SyncInfo(on_wait=[w], on_update=[])
                        new.append(nop)
                    inst.sync_info = mybir.SyncInfo(
                        on_wait=waits[-max_waits:], on_update=list(si.on_update))
                    changed = True
                new.append(inst)
            if changed:
                blk.instructions = new


def _build_program():
    import concourse.bass as bass
    import concourse.tile as tile
    from concourse import mybir

    nc = bass.Bass("TRN2", target_bir_lowering=False, debug=False)

    ext = nc.dram_tensor("ext", [128, EXT_COLS], mybir.dt.bfloat16,
                         kind="ExternalInput").ap()
    out = nc.dram_tensor("out", [N_BINS, F_PER_CORE], mybir.dt.float32,
                         kind="ExternalOutput").ap()

    with tile.TileContext(nc) as tc:
        with (
            tc.tile_pool(name="const", bufs=1) as const,
            tc.tile_pool(name="psum", bufs=4, space="PSUM") as psum,
            tc.tile_pool(name="tmp", bufs=4) as tmp,
            tc.tile_pool(name="outp", bufs=1) as outp,
        ):
            xt = const.tile([128, EXT_COLS], mybir.dt.bfloat16)
            nc.sync.dma_start(xt[:], ext)

            o = outp.tile([N_BINS, F_PER_CORE], mybir.dt.float32)
            for fb in range(N_FB):
                ps_a = psum.tile([N_BINS, 512], mybir.dt.float32)
                ps_b = psum.tile([N_BINS, 512], mybir.dt.float32)
                base = fb * 4 * 512
                for kc in range(N_KC):
                    rhs = xt[:, base + kc: base + kc + 2048: 4]
                    la = xt[:, X_COLS + kc * 168: X_COLS + kc * 168 + N_BINS]
                    lb = xt[:, X_COLS + kc * 168 + N_BINS:
                            X_COLS + kc * 168 + 2 * N_BINS]
                    nc.tensor.matmul(ps_a[:], la, rhs,
                                     start=(kc == 0), stop=(kc == N_KC - 1))
                    nc.tensor.matmul(ps_b[:], lb, rhs,
                                     start=(kc == 0), stop=(kc == N_KC - 1))
                # a^2 + b^2 on DVE only (PSUM allows one PSUM operand per op)
                cpa = tmp.tile([N_BINS, 512], mybir.dt.float32, tag="cpa")
                nc.vector.tensor_copy(cpa[:], ps_a[:])
                sqa = tmp.tile([N_BINS, 512], mybir.dt.float32, tag="sqa")
                nc.vector.tensor_mul(sqa[:], ps_a[:], cpa[:])
                cpb = tmp.tile([N_BINS, 512], mybir.dt.float32, tag="cpb")
                nc.vector.tensor_copy(cpb[:], ps_b[:])
                sqb = tmp.tile([N_BINS, 512], mybir.dt.float32, tag="sqb")
                nc.vector.tensor_mul(sqb[:], ps_b[:], cpb[:])
                nc.vector.tensor_add(o[:, fb * 512:(fb + 1) * 512],
                                     sqa[:], sqb[:])
            nc.gpsimd.dma_start(out[:], o[:])

    _split_multi_waits(nc, mybir)
    return nc


def _get_program():
    global _PROGRAM
    if _PROGRAM is None:
        _PROGRAM = _build_program()
    return _PROGRAM


def _host_prep(x, wcos, wsin, kr, ki):
    """Fold the CQT kernels; shard, cast, and lay out the waveform."""
    kr64 = np.asarray(kr, dtype=np.float64)
    ki64 = np.asarray(ki, dtype=np.float64)
    wc64 = np.asarray(wcos, dtype=np.float64)
    ws64 = np.asarray(wsin, dtype=np.float64)
    a = kr64 @ wc64 - ki64 @ ws64            # [84, 2048]
    b = kr64 @ ws64 + ki64 @ wc64            # [84, 2048]
    abt = np.concatenate([a, b], axis=0).T   # [2048, 168]
    # [128, 2688]: ab_pack[p, kc*168+j] = abt[kc*128+p, j]
    ab_pack = np.ascontiguousarray(
        abt.reshape(N_KC, 128, 2 * N_BINS).transpose(1, 0, 2)
    ).reshape(128, AB_COLS).astype(ml_dtypes.bfloat16)

    x = np.asarray(x, dtype=np.float32)
    x_pad = np.zeros((N_CORES - 1) * CORE_STRIDE + SHARD_LEN, dtype=np.float32)
    x_pad[:T_SAMPLES] = x
    x_bf = x_pad.astype(ml_dtypes.bfloat16)
    exts = []
    for c in range(N_CORES):
        shard = x_bf[c * CORE_STRIDE: c * CORE_STRIDE + SHARD_LEN]
        ext = np.empty((128, EXT_COLS), dtype=ml_dtypes.bfloat16)
        ext[:, :X_COLS] = shard.reshape(X_COLS, 128).T
        ext[:, X_COLS:] = ab_pack
        exts.append(ext)
    return exts


_LAST_RESULTS = None  # BassKernelResults of the most recent run (for profiling)


def kernel(x, wcos, wsin, kr, ki):
    global _LAST_RESULTS
    from concourse.bass_utils import run_bass_kernel_spmd

    exts = _host_prep(x, wcos, wsin, kr, ki)
    nc = _get_program()
    in_maps = [{"ext": exts[c]} for c in range(N_CORES)]
    res = run_bass_kernel_spmd(nc, in_maps, core_ids=list(range(N_CORES)))
    _LAST_RESULTS = res
    full = np.concatenate([res.results[c]["out"] for c in range(N_CORES)], axis=1)
    return np.sqrt(full[None, :, :N_FRAMES]).astype(np.float32)


# revision 17
# speedup vs baseline: 1.5791x; 1.5791x over previous
"""Trainium2 Bass kernel: CQT (constant-Q transform) of 2^23 audio samples.

Reference math (jax):
    frames[f, n] = x[f*HOP + n]                  HOP=512, fftLen=2048
    four_r = frames @ wcos.T ; four_i = frames @ wsin.T
    cqt_r  = kr @ four_r - ki @ four_i
    cqt_i  = kr @ four_i + ki @ four_r
    out    = sqrt(cqt_r**2 + cqt_i**2)           # [1, 84, n_frames]

Folded on the host (exact algebra, tiny matrices):
    A = kr@wcos - ki@wsin,  B = kr@wsin + ki@wcos      (each [84, 2048])
    out = sqrt((A @ frames.T)**2 + (B @ frames.T)**2)

Device strategy (8-way shard along the frame axis; kernels replicated):
  - 2048 frames per core.  The bf16 x-shard is laid out host-side so that
    the matmul's moving operand is always a CONTIGUOUS column range: with
    xt[p, c] = x[c*128 + p], contraction chunk kc = 4a + r of frame f needs
    column 4*(f+a) + r, so columns are stored deinterleaved by (frame-block,
    r-plane).  A.T/B.T chunks ride the same DRAM tensor.  (A strided rhs AP
    halves the PE's bf16 stream rate - measured 452 -> 216 ns per matmul.)
  - input DMA is split per frame-block so fb0's matmuls start ~3us in;
    4 fb x 16 kc x {A,B} matmuls accumulate into 8 PSUM banks; a^2+b^2 on
    VectorE; one SWDGE DMA out.  sqrt on the host (monotone, exact).
  - a post-pass splits multi-wait instructions: this walrus build encodes
    at most ONE semaphore wait per instruction.
"""

import sys

if "/opt/trn_rl_repo" not in sys.path:
    sys.path.insert(0, "/opt/trn_rl_repo")

import numpy as np
import ml_dtypes

HOP = 512
FFTLEN = 2048
N_BINS = 84
T_SAMPLES = 8388608
N_FRAMES = (T_SAMPLES - FFTLEN) // HOP + 1  # 16381
N_CORES = 8
F_PER_CORE = 2048                 # frames computed per core (3 junk at the end)
X_COLS_TOTAL = 8204               # sample columns actually needed per core
SHARD_LEN = X_COLS_TOTAL * 128    # 1050112 samples per core
CORE_STRIDE = F_PER_CORE * HOP    # 1048576 samples between shard starts
N_KC = FFTLEN // 128              # 16 contraction chunks
N_FB = F_PER_CORE // 512          # 4 frame blocks of 512 frames
PLANE_COLS = 515                  # columns per r-plane per frame block
FB_COLS = 4 * PLANE_COLS          # 2060
AB_COLS = N_KC * 2 * N_BINS       # 2688 columns holding A.T/B.T chunks
EXT_COLS = AB_COLS + N_FB * FB_COLS  # 10928

_PROGRAM = None


def _split_multi_waits(nc, mybir, max_waits=1):
    """This walrus build encodes at most one sem wait per instruction; move
    extra waits onto injected same-engine NoOps right before the instruction."""
    ctr = 0
    for f in nc.m.functions:
        for blk in f.blocks:
            il = list(blk.instructions)
            new = []
            changed = False
            for inst in il:
                si = getattr(inst, "sync_info", None)
                if si is not None and len(si.on_wait) > max_waits:
                    waits = list(si.on_wait)
                    for w in waits[:-max_waits]:
                        nop = mybir.InstNoOp(name=f"I-waitfix-{ctr}", ins=[], outs=[])
                        ctr += 1
                        nop.engine = inst.engine
                        nop.sync_info = mybir.SyncInfo(on_wait=[w], on_update=[])
                        new.append(nop)
                    inst.sync_info = mybir.SyncInfo(
                        on_wait=waits[-max_waits:], on_update=list(si.on_update))
                    changed = True
                new.append(inst)
            if changed:
                blk.instructions = new


def _build_program():
    import concourse.bass as bass
    import concourse.tile as tile
    from concourse import mybir

    nc = bass.Bass("TRN2", target_bir_lowering=False, debug=False)

    ext = nc.dram_tensor("ext", [128, EXT_COLS], mybir.dt.bfloat16,
                         kind="ExternalInput").ap()
    out = nc.dram_tensor("out", [N_BINS, F_PER_CORE], mybir.dt.float32,
                         kind="ExternalOutput").ap()

    with tile.TileContext(nc) as tc:
        with (
            tc.tile_pool(name="const", bufs=1) as const,
            tc.tile_pool(name="psum", bufs=4, space="PSUM") as psum,
            tc.tile_pool(name="tmp", bufs=4) as tmp,
            tc.tile_pool(name="outp", bufs=1) as outp,
        ):
            xt = const.tile([128, EXT_COLS], mybir.dt.bfloat16)
            # chunked input: AB + frame block 0 first, then fb1..fb3, so fb0
            # matmuls start as soon as the first chunk lands
            nc.sync.dma_start(xt[:, :AB_COLS + FB_COLS],
                              ext[:, :AB_COLS + FB_COLS])
            for fb in range(1, N_FB):
                lo = AB_COLS + fb * FB_COLS
                nc.sync.dma_start(xt[:, lo:lo + FB_COLS],
                                  ext[:, lo:lo + FB_COLS])

            o = outp.tile([N_BINS, F_PER_CORE], mybir.dt.float32)
            for fb in range(N_FB):
                ps_a = psum.tile([N_BINS, 512], mybir.dt.float32)
                ps_b = psum.tile([N_BINS, 512], mybir.dt.float32)
                fb_lo = AB_COLS + fb * FB_COLS
                for kc in range(N_KC):
                    a_, r_ = divmod(kc, 4)
                    lo = fb_lo + r_ * PLANE_COLS + a_
                    rhs = xt[:, lo:lo + 512]
                    la = xt[:, kc * 168: kc * 168 + N_BINS]
                    lb = xt[:, kc * 168 + N_BINS: kc * 168 + 2 * N_BINS]
                    nc.tensor.matmul(ps_a[:], la, rhs,
                                     start=(kc == 0), stop=(kc == N_KC - 1))
                    nc.tensor.matmul(ps_b[:], lb, rhs,
                                     start=(kc == 0), stop=(kc == N_KC - 1))
                # a^2 + b^2 on DVE only (one PSUM operand per op)
                cpa = tmp.tile([N_BINS, 512], mybir.dt.float32, tag="cpa")
                nc.vector.tensor_copy(cpa[:], ps_a[:])
                sqa = tmp.tile([N_BINS, 512], mybir.dt.float32, tag="sqa")
                nc.vector.tensor_mul(sqa[:], ps_a[:], cpa[:])
                cpb = tmp.tile([N_BINS, 512], mybir.dt.float32, tag="cpb")
                nc.vector.tensor_copy(cpb[:], ps_b[:])
                sqb = tmp.tile([N_BINS, 512], mybir.dt.float32, tag="sqb")
                nc.vector.tensor_mul(sqb[:], ps_b[:], cpb[:])
                nc.vector.tensor_add(o[:, fb * 512:(fb + 1) * 512],
                                     sqa[:], sqb[:])
            nc.gpsimd.dma_start(out[:], o[:])

    _split_multi_waits(nc, mybir)
    return nc


def _get_program():
    global _PROGRAM
    if _PROGRAM is None:
        _PROGRAM = _build_program()
    return _PROGRAM


def _host_prep(x, wcos, wsin, kr, ki):
    """Fold the CQT kernels; shard, cast, and lay out the waveform."""
    kr64 = np.asarray(kr, dtype=np.float64)
    ki64 = np.asarray(ki, dtype=np.float64)
    wc64 = np.asarray(wcos, dtype=np.float64)
    ws64 = np.asarray(wsin, dtype=np.float64)
    a = kr64 @ wc64 - ki64 @ ws64            # [84, 2048]
    b = kr64 @ ws64 + ki64 @ wc64            # [84, 2048]
    abt = np.concatenate([a, b], axis=0).T   # [2048, 168]
    # [128, 2688]: ab_pack[p, kc*168+j] = abt[kc*128+p, j]
    ab_pack = np.ascontiguousarray(
        abt.reshape(N_KC, 128, 2 * N_BINS).transpose(1, 0, 2)
    ).reshape(128, AB_COLS).astype(ml_dtypes.bfloat16)

    x = np.asarray(x, dtype=np.float32)
    x_pad = np.zeros((N_CORES - 1) * CORE_STRIDE + SHARD_LEN, dtype=np.float32)
    x_pad[:T_SAMPLES] = x
    x_bf = x_pad.astype(ml_dtypes.bfloat16)
    exts = []
    for c in range(N_CORES):
        shard = x_bf[c * CORE_STRIDE: c * CORE_STRIDE + SHARD_LEN]
        # zz[j, r, p] = x[(4j+r)*128 + p]
        zz = shard.reshape(X_COLS_TOTAL // 4, 4, 128)
        ext = np.empty((128, EXT_COLS), dtype=ml_dtypes.bfloat16)
        ext[:, :AB_COLS] = ab_pack
        for fb in range(N_FB):
            lo = AB_COLS + fb * FB_COLS
            for r in range(4):
                ext[:, lo + r * PLANE_COLS: lo + (r + 1) * PLANE_COLS] = (
                    zz[fb * 512: fb * 512 + PLANE_COLS, r, :].T)
        exts.append(ext)
    return exts


_LAST_RESULTS = None  # BassKernelResults of the most recent run (for profiling)


def kernel(x, wcos, wsin, kr, ki):
    global _LAST_RESULTS
    from concourse.bass_utils import run_bass_kernel_spmd

    exts = _host_prep(x, wcos, wsin, kr, ki)
    nc = _get_program()
    in_maps = [{"ext": exts[c]} for c in range(N_CORES)]
    res = run_bass_kernel_spmd(nc, in_maps, core_ids=list(range(N_CORES)))
    _LAST_RESULTS = res
    full = np.concatenate([res.results[c]["out"] for c in range(N_CORES)], axis=1)
    return np.sqrt(full[None, :, :N_FRAMES]).astype(np.float32)


# revision 18
# speedup vs baseline: 1.7065x; 1.0807x over previous
"""Trainium2 Bass kernel: CQT (constant-Q transform) of 2^23 audio samples.

Reference math (jax):
    frames[f, n] = x[f*HOP + n]                  HOP=512, fftLen=2048
    four_r = frames @ wcos.T ; four_i = frames @ wsin.T
    cqt_r  = kr @ four_r - ki @ four_i
    cqt_i  = kr @ four_i + ki @ four_r
    out    = sqrt(cqt_r**2 + cqt_i**2)           # [1, 84, n_frames]

Folded on the host (exact algebra, tiny matrices):
    A = kr@wcos - ki@wsin,  B = kr@wsin + ki@wcos      (each [84, 2048])
    out = sqrt((A @ frames.T)**2 + (B @ frames.T)**2)

Device strategy (8-way shard along the frame axis; kernels replicated):
  - 2048 frames per core.  The bf16 x-shard is laid out host-side so that
    the matmul's moving operand is always a CONTIGUOUS column range: with
    xt[p, c] = x[c*128 + p], contraction chunk kc = 4a + r of frame f needs
    column 4*(f+a) + r, so columns are stored deinterleaved by (frame-block,
    r-plane).  A.T/B.T chunks ride the same DRAM tensor.  (A strided rhs AP
    halves the PE's bf16 stream rate - measured 452 -> 216 ns per matmul.)
  - input DMA is split per frame-block so fb0's matmuls start ~3us in;
    4 fb x 16 kc x {A,B} matmuls accumulate into 8 PSUM banks; a^2+b^2 on
    VectorE; one SWDGE DMA out.  sqrt on the host (monotone, exact).
  - a post-pass splits multi-wait instructions: this walrus build encodes
    at most ONE semaphore wait per instruction.
"""

import sys

if "/opt/trn_rl_repo" not in sys.path:
    sys.path.insert(0, "/opt/trn_rl_repo")

import numpy as np
import ml_dtypes

HOP = 512
FFTLEN = 2048
N_BINS = 84
T_SAMPLES = 8388608
N_FRAMES = (T_SAMPLES - FFTLEN) // HOP + 1  # 16381
N_CORES = 8
F_PER_CORE = 2048                 # frames computed per core (3 junk at the end)
X_COLS_TOTAL = 8204               # sample columns actually needed per core
SHARD_LEN = X_COLS_TOTAL * 128    # 1050112 samples per core
CORE_STRIDE = F_PER_CORE * HOP    # 1048576 samples between shard starts
N_KC = FFTLEN // 128              # 16 contraction chunks
N_FB = F_PER_CORE // 512          # 4 frame blocks of 512 frames
PLANE_COLS = 515                  # columns per r-plane per frame block
FB_COLS = 4 * PLANE_COLS          # 2060
AB_COLS = N_KC * 2 * N_BINS       # 2688 columns holding A.T/B.T chunks
EXT_COLS = AB_COLS + N_FB * FB_COLS  # 10928

_PROGRAM = None


def _split_multi_waits(nc, mybir, max_waits=1):
    """This walrus build encodes at most one sem wait per instruction; move
    extra waits onto injected same-engine NoOps right before the instruction."""
    ctr = 0
    for f in nc.m.functions:
        for blk in f.blocks:
            il = list(blk.instructions)
            new = []
            changed = False
            for inst in il:
                si = getattr(inst, "sync_info", None)
                if si is not None and len(si.on_wait) > max_waits:
                    waits = list(si.on_wait)
                    for w in waits[:-max_waits]:
                        nop = mybir.InstNoOp(name=f"I-waitfix-{ctr}", ins=[], outs=[])
                        ctr += 1
                        nop.engine = inst.engine
                        nop.sync_info = mybir.SyncInfo(on_wait=[w], on_update=[])
                        new.append(nop)
                    inst.sync_info = mybir.SyncInfo(
                        on_wait=waits[-max_waits:], on_update=list(si.on_update))
                    changed = True
                new.append(inst)
            if changed:
                blk.instructions = new


def _build_program():
    import concourse.bass as bass
    import concourse.tile as tile
    from concourse import mybir

    nc = bass.Bass("TRN2", target_bir_lowering=False, debug=False)

    ext = nc.dram_tensor("ext", [128, EXT_COLS], mybir.dt.bfloat16,
                         kind="ExternalInput").ap()
    out = nc.dram_tensor("out", [N_BINS, F_PER_CORE], mybir.dt.float32,
                         kind="ExternalOutput").ap()

    with tile.TileContext(nc) as tc:
        with (
            tc.tile_pool(name="const", bufs=1) as const,
            tc.tile_pool(name="psum", bufs=4, space="PSUM") as psum,
            tc.tile_pool(name="tmp", bufs=4) as tmp,
            tc.tile_pool(name="outp", bufs=1) as outp,
        ):
            xt = const.tile([128, EXT_COLS], mybir.dt.bfloat16)
            # chunked input: AB kernels first, then fb0 plane-by-plane (the
            # r-major kc order below starts matmuls after AB + one plane),
            # then fb1..fb3 whole
            nc.sync.dma_start(xt[:, :AB_COLS], ext[:, :AB_COLS])
            for r in range(4):
                lo = AB_COLS + r * PLANE_COLS
                nc.sync.dma_start(xt[:, lo:lo + PLANE_COLS],
                                  ext[:, lo:lo + PLANE_COLS])
            for fb in range(1, N_FB):
                lo = AB_COLS + fb * FB_COLS
                nc.sync.dma_start(xt[:, lo:lo + FB_COLS],
                                  ext[:, lo:lo + FB_COLS])

            o = outp.tile([N_BINS, F_PER_CORE], mybir.dt.float32)
            for fb in range(N_FB):
                ps_a = psum.tile([N_BINS, 512], mybir.dt.float32)
                ps_b = psum.tile([N_BINS, 512], mybir.dt.float32)
                fb_lo = AB_COLS + fb * FB_COLS
                kcs = [4 * a_ + r_ for r_ in range(4) for a_ in range(4)]
                for i, kc in enumerate(kcs):
                    a_, r_ = divmod(kc, 4)
                    lo = fb_lo + r_ * PLANE_COLS + a_
                    rhs = xt[:, lo:lo + 512]
                    la = xt[:, kc * 168: kc * 168 + N_BINS]
                    lb = xt[:, kc * 168 + N_BINS: kc * 168 + 2 * N_BINS]
                    nc.tensor.matmul(ps_a[:], la, rhs,
                                     start=(i == 0), stop=(i == N_KC - 1))
                    nc.tensor.matmul(ps_b[:], lb, rhs,
                                     start=(i == 0), stop=(i == N_KC - 1))
                # a^2 + b^2 on DVE only (one PSUM operand per op)
                cpa = tmp.tile([N_BINS, 512], mybir.dt.float32, tag="cpa")
                nc.vector.tensor_copy(cpa[:], ps_a[:])
                sqa = tmp.tile([N_BINS, 512], mybir.dt.float32, tag="sqa")
                nc.vector.tensor_mul(sqa[:], ps_a[:], cpa[:])
                cpb = tmp.tile([N_BINS, 512], mybir.dt.float32, tag="cpb")
                nc.vector.tensor_copy(cpb[:], ps_b[:])
                sqb = tmp.tile([N_BINS, 512], mybir.dt.float32, tag="sqb")
                nc.vector.tensor_mul(sqb[:], ps_b[:], cpb[:])
                nc.vector.tensor_add(o[:, fb * 512:(fb + 1) * 512],
                                     sqa[:], sqb[:])
                nc.sync.dma_start(out[:, fb * 512:(fb + 1) * 512],
                                  o[:, fb * 512:(fb + 1) * 512])

    _split_multi_waits(nc, mybir)
    return nc


def _get_program():
    global _PROGRAM
    if _PROGRAM is None:
        _PROGRAM = _build_program()
    return _PROGRAM


def _host_prep(x, wcos, wsin, kr, ki):
    """Fold the CQT kernels; shard, cast, and lay out the waveform."""
    kr64 = np.asarray(kr, dtype=np.float64)
    ki64 = np.asarray(ki, dtype=np.float64)
    wc64 = np.asarray(wcos, dtype=np.float64)
    ws64 = np.asarray(wsin, dtype=np.float64)
    a = kr64 @ wc64 - ki64 @ ws64            # [84, 2048]
    b = kr64 @ ws64 + ki64 @ wc64            # [84, 2048]
    abt = np.concatenate([a, b], axis=0).T   # [2048, 168]
    # [128, 2688]: ab_pack[p, kc*168+j] = abt[kc*128+p, j]
    ab_pack = np.ascontiguousarray(
        abt.reshape(N_KC, 128, 2 * N_BINS).transpose(1, 0, 2)
    ).reshape(128, AB_COLS).astype(ml_dtypes.bfloat16)

    x = np.asarray(x, dtype=np.float32)
    x_pad = np.zeros((N_CORES - 1) * CORE_STRIDE + SHARD_LEN, dtype=np.float32)
    x_pad[:T_SAMPLES] = x
    x_bf = x_pad.astype(ml_dtypes.bfloat16)
    exts = []
    for c in range(N_CORES):
        shard = x_bf[c * CORE_STRIDE: c * CORE_STRIDE + SHARD_LEN]
        # zz[j, r, p] = x[(4j+r)*128 + p]
        zz = shard.reshape(X_COLS_TOTAL // 4, 4, 128)
        ext = np.empty((128, EXT_COLS), dtype=ml_dtypes.bfloat16)
        ext[:, :AB_COLS] = ab_pack
        for fb in range(N_FB):
            lo = AB_COLS + fb * FB_COLS
            for r in range(4):
                ext[:, lo + r * PLANE_COLS: lo + (r + 1) * PLANE_COLS] = (
                    zz[fb * 512: fb * 512 + PLANE_COLS, r, :].T)
        exts.append(ext)
    return exts


_LAST_RESULTS = None  # BassKernelResults of the most recent run (for profiling)


def kernel(x, wcos, wsin, kr, ki):
    global _LAST_RESULTS
    from concourse.bass_utils import run_bass_kernel_spmd

    exts = _host_prep(x, wcos, wsin, kr, ki)
    nc = _get_program()
    in_maps = [{"ext": exts[c]} for c in range(N_CORES)]
    res = run_bass_kernel_spmd(nc, in_maps, core_ids=list(range(N_CORES)))
    _LAST_RESULTS = res
    full = np.concatenate([res.results[c]["out"] for c in range(N_CORES)], axis=1)
    return np.sqrt(full[None, :, :N_FRAMES]).astype(np.float32)


# revision 21
# speedup vs baseline: 1.7806x; 1.0435x over previous
"""Trainium2 Bass kernel: CQT (constant-Q transform) of 2^23 audio samples.

Reference math (jax):
    frames[f, n] = x[f*HOP + n]                  HOP=512, fftLen=2048
    four_r = frames @ wcos.T ; four_i = frames @ wsin.T
    cqt_r  = kr @ four_r - ki @ four_i
    cqt_i  = kr @ four_i + ki @ four_r
    out    = sqrt(cqt_r**2 + cqt_i**2)           # [1, 84, n_frames]

Folded on the host (exact algebra, tiny matrices):
    A = kr@wcos - ki@wsin,  B = kr@wsin + ki@wcos      (each [84, 2048])
    out = sqrt((A @ frames.T)**2 + (B @ frames.T)**2)

Device strategy (8-way shard along the frame axis; kernels replicated):
  - 2048 frames per core.  The bf16 x-shard is laid out host-side so that
    the matmul's moving operand is always a CONTIGUOUS column range: with
    xt[p, c] = x[c*128 + p], contraction chunk kc = 4a + r of frame f needs
    column 4*(f+a) + r, so columns are stored deinterleaved by (frame-block,
    r-plane).  A.T/B.T chunks ride the same DRAM tensor.  (A strided rhs AP
    halves the PE's bf16 stream rate - measured 452 -> 216 ns per matmul.)
  - input DMA is split per frame-block so fb0's matmuls start ~3us in;
    4 fb x 16 kc x {A,B} matmuls accumulate into 8 PSUM banks; a^2+b^2 on
    VectorE; one SWDGE DMA out.  sqrt on the host (monotone, exact).
  - a post-pass splits multi-wait instructions: this walrus build encodes
    at most ONE semaphore wait per instruction.
"""

import sys

if "/opt/trn_rl_repo" not in sys.path:
    sys.path.insert(0, "/opt/trn_rl_repo")

import numpy as np
import ml_dtypes

HOP = 512
FFTLEN = 2048
N_BINS = 84
T_SAMPLES = 8388608
N_FRAMES = (T_SAMPLES - FFTLEN) // HOP + 1  # 16381
N_CORES = 8
F_PER_CORE = 2048                 # frames computed per core (3 junk at the end)
X_COLS_TOTAL = 8204               # sample columns actually needed per core
SHARD_LEN = X_COLS_TOTAL * 128    # 1050112 samples per core
CORE_STRIDE = F_PER_CORE * HOP    # 1048576 samples between shard starts
N_KC = FFTLEN // 128              # 16 contraction chunks
N_FB = F_PER_CORE // 512          # 4 frame blocks of 512 frames
PLANE_COLS = 515                  # columns per r-plane per frame block
FB_COLS = 4 * PLANE_COLS          # 2060
AB_R_COLS = 4 * 2 * N_BINS        # 672: the 4 kc-chunks of A.T/B.T for one r
CH_COLS = AB_R_COLS + PLANE_COLS  # 1187: one [AB_r | fb0 plane r] chunk
FB1_LO = 4 * CH_COLS              # 4748: start of the fb1..fb3 blocks
AB_COLS = N_KC * 2 * N_BINS       # 2688 columns holding A.T/B.T chunks
EXT_COLS = FB1_LO + (N_FB - 1) * FB_COLS  # 10928

_PROGRAM = None


def _split_multi_waits(nc, mybir, max_waits=1):
    """This walrus build encodes at most one sem wait per instruction; move
    extra waits onto injected same-engine NoOps right before the instruction."""
    ctr = 0
    for f in nc.m.functions:
        for blk in f.blocks:
            il = list(blk.instructions)
            new = []
            changed = False
            for inst in il:
                si = getattr(inst, "sync_info", None)
                if si is not None and len(si.on_wait) > max_waits:
                    waits = list(si.on_wait)
                    for w in waits[:-max_waits]:
                        nop = mybir.InstNoOp(name=f"I-waitfix-{ctr}", ins=[], outs=[])
                        ctr += 1
                        nop.engine = inst.engine
                        nop.sync_info = mybir.SyncInfo(on_wait=[w], on_update=[])
                        new.append(nop)
                    inst.sync_info = mybir.SyncInfo(
                        on_wait=waits[-max_waits:], on_update=list(si.on_update))
                    changed = True
                new.append(inst)
            if changed:
                blk.instructions = new


def _build_program():
    import concourse.bass as bass
    import concourse.tile as tile
    from concourse import mybir

    nc = bass.Bass("TRN2", target_bir_lowering=False, debug=False)

    ext = nc.dram_tensor("ext", [128, EXT_COLS], mybir.dt.bfloat16,
                         kind="ExternalInput").ap()
    out = nc.dram_tensor("out", [N_BINS, F_PER_CORE], mybir.dt.float32,
                         kind="ExternalOutput").ap()

    with tile.TileContext(nc) as tc:
        with (
            tc.tile_pool(name="const", bufs=1) as const,
            tc.tile_pool(name="psum", bufs=4, space="PSUM") as psum,
            tc.tile_pool(name="tmp", bufs=4) as tmp,
            tc.tile_pool(name="outp", bufs=1) as outp,
        ):
            xt = const.tile([128, EXT_COLS], mybir.dt.bfloat16)
            # chunked input: [AB_r | fb0 plane r] per r (the r-major kc order
            # below starts matmuls after one 300KB chunk), then fb1..fb3
            for r in range(4):
                lo = r * CH_COLS
                nc.sync.dma_start(xt[:, lo:lo + CH_COLS],
                                  ext[:, lo:lo + CH_COLS])
            for fb in range(1, N_FB):
                lo = FB1_LO + (fb - 1) * FB_COLS
                nc.sync.dma_start(xt[:, lo:lo + FB_COLS],
                                  ext[:, lo:lo + FB_COLS])

            o = outp.tile([N_BINS, F_PER_CORE], mybir.dt.float32)
            for fb in range(N_FB):
                ps_a = psum.tile([N_BINS, 512], mybir.dt.float32)
                ps_b = psum.tile([N_BINS, 512], mybir.dt.float32)
                for i, (r_, a_) in enumerate(
                        (r_, a_) for r_ in range(4) for a_ in range(4)):
                    if fb == 0:
                        lo = r_ * CH_COLS + AB_R_COLS + a_
                    else:
                        lo = FB1_LO + (fb - 1) * FB_COLS + r_ * PLANE_COLS + a_
                    rhs = xt[:, lo:lo + 512]
                    ab = r_ * CH_COLS + a_ * 2 * N_BINS
                    la = xt[:, ab: ab + N_BINS]
                    lb = xt[:, ab + N_BINS: ab + 2 * N_BINS]
                    nc.tensor.matmul(ps_a[:], la, rhs,
                                     start=(i == 0), stop=(i == N_KC - 1))
                    nc.tensor.matmul(ps_b[:], lb, rhs,
                                     start=(i == 0), stop=(i == N_KC - 1))
                # a^2 + b^2: squares on ScalarE (parallel to DVE), add on DVE
                sqa = tmp.tile([N_BINS, 512], mybir.dt.float32, tag="sqa")
                nc.scalar.square(sqa[:], ps_a[:])
                sqb = tmp.tile([N_BINS, 512], mybir.dt.float32, tag="sqb")
                nc.scalar.square(sqb[:], ps_b[:])
                nc.vector.tensor_add(o[:, fb * 512:(fb + 1) * 512],
                                     sqa[:], sqb[:])
                nc.sync.dma_start(out[:, fb * 512:(fb + 1) * 512],
                                  o[:, fb * 512:(fb + 1) * 512])

    _split_multi_waits(nc, mybir)
    return nc


def _get_program():
    global _PROGRAM
    if _PROGRAM is None:
        _PROGRAM = _build_program()
    return _PROGRAM


def _host_prep(x, wcos, wsin, kr, ki):
    """Fold the CQT kernels; shard, cast, and lay out the waveform."""
    kr64 = np.asarray(kr, dtype=np.float64)
    ki64 = np.asarray(ki, dtype=np.float64)
    wc64 = np.asarray(wcos, dtype=np.float64)
    ws64 = np.asarray(wsin, dtype=np.float64)
    a = kr64 @ wc64 - ki64 @ ws64            # [84, 2048]
    b = kr64 @ ws64 + ki64 @ wc64            # [84, 2048]
    abt = np.concatenate([a, b], axis=0).T   # [2048, 168]
    # abkc[kc][p, j] = abt[kc*128+p, j]
    abkc = abt.reshape(N_KC, 128, 2 * N_BINS).astype(ml_dtypes.bfloat16)

    x = np.asarray(x, dtype=np.float32)
    x_pad = np.zeros((N_CORES - 1) * CORE_STRIDE + SHARD_LEN, dtype=np.float32)
    x_pad[:T_SAMPLES] = x
    x_bf = x_pad.astype(ml_dtypes.bfloat16)
    exts = []
    for c in range(N_CORES):
        shard = x_bf[c * CORE_STRIDE: c * CORE_STRIDE + SHARD_LEN]
        # zz[j, r, p] = x[(4j+r)*128 + p]
        zz = shard.reshape(X_COLS_TOTAL // 4, 4, 128)
        ext = np.empty((128, EXT_COLS), dtype=ml_dtypes.bfloat16)
        for r in range(4):
            lo = r * CH_COLS
            for a_ in range(4):
                ext[:, lo + a_ * 2 * N_BINS: lo + (a_ + 1) * 2 * N_BINS] = (
                    abkc[4 * a_ + r])
            ext[:, lo + AB_R_COLS: lo + CH_COLS] = zz[:PLANE_COLS, r, :].T
        for fb in range(1, N_FB):
            lo = FB1_LO + (fb - 1) * FB_COLS
            for r in range(4):
                ext[:, lo + r * PLANE_COLS: lo + (r + 1) * PLANE_COLS] = (
                    zz[fb * 512: fb * 512 + PLANE_COLS, r, :].T)
        exts.append(ext)
    return exts


_LAST_RESULTS = None  # BassKernelResults of the most recent run (for profiling)


def kernel(x, wcos, wsin, kr, ki):
    global _LAST_RESULTS
    from concourse.bass_utils import run_bass_kernel_spmd

    exts = _host_prep(x, wcos, wsin, kr, ki)
    nc = _get_program()
    in_maps = [{"ext": exts[c]} for c in range(N_CORES)]
    res = run_bass_kernel_spmd(nc, in_maps, core_ids=list(range(N_CORES)))
    _LAST_RESULTS = res
    full = np.concatenate([res.results[c]["out"] for c in range(N_CORES)], axis=1)
    return np.sqrt(full[None, :, :N_FRAMES]).astype(np.float32)


# revision 28
# speedup vs baseline: 2.2705x; 1.2751x over previous
"""Trainium2 Bass kernel: CQT (constant-Q transform) of 2^23 audio samples.

Reference math (jax):
    frames[f, n] = x[f*HOP + n]                  HOP=512, fftLen=2048
    four_r = frames @ wcos.T ; four_i = frames @ wsin.T
    cqt_r  = kr @ four_r - ki @ four_i
    cqt_i  = kr @ four_i + ki @ four_r
    out    = sqrt(cqt_r**2 + cqt_i**2)           # [1, 84, n_frames]

Folded on the host (exact algebra, tiny matrices):
    A = kr@wcos - ki@wsin,  B = kr@wsin + ki@wcos      (each [84, 2048])
    out = sqrt((A @ frames.T)**2 + (B @ frames.T)**2)

Device strategy (8-way shard along the frame axis; kernels replicated):
  - 2048 frames per core.  The bf16 x-shard is laid out host-side so that
    the matmul's moving operand is always a CONTIGUOUS column range: with
    xt[p, c] = x[c*128 + p], contraction chunk kc = 4a + r of frame f needs
    column 4*(f+a) + r, so columns are stored deinterleaved by (frame-block,
    r-plane).  A.T/B.T chunks ride the same DRAM tensor.  (A strided rhs AP
    halves the PE's bf16 stream rate - measured 452 -> 216 ns per matmul.)
  - input DMA is split per frame-block so fb0's matmuls start ~3us in;
    4 fb x 16 kc x {A,B} matmuls accumulate into 8 PSUM banks; a^2+b^2 on
    VectorE; one SWDGE DMA out.  sqrt on the host (monotone, exact).
  - a post-pass splits multi-wait instructions: this walrus build encodes
    at most ONE semaphore wait per instruction.
"""

import sys

if "/opt/trn_rl_repo" not in sys.path:
    sys.path.insert(0, "/opt/trn_rl_repo")

import numpy as np
import ml_dtypes

HOP = 512
FFTLEN = 2048
N_BINS = 84
T_SAMPLES = 8388608
N_FRAMES = (T_SAMPLES - FFTLEN) // HOP + 1  # 16381
N_CORES = 8
F_PER_CORE = 2048                 # frames computed per core (3 junk at the end)
X_COLS_TOTAL = 8204               # sample columns actually needed per core
SHARD_LEN = X_COLS_TOTAL * 128    # 1050112 samples per core
CORE_STRIDE = F_PER_CORE * HOP    # 1048576 samples between shard starts
N_KC = FFTLEN // 128              # 16 contraction chunks
N_FB = F_PER_CORE // 512          # 4 frame blocks of 512 frames
PLANE_COLS = 515                  # columns per r-plane per frame block
FB_COLS = 4 * PLANE_COLS          # 2060
AB_R_COLS = 4 * 2 * N_BINS        # 672: the 4 kc-chunks of A.T/B.T for one r
CH_COLS = AB_R_COLS + PLANE_COLS  # 1187: one [AB_r | fb0 plane r] chunk
FB1_LO = 4 * CH_COLS              # 4748: start of the fb1..fb3 blocks
AB_COLS = N_KC * 2 * N_BINS       # 2688 columns holding A.T/B.T chunks
EXT_COLS = FB1_LO + (N_FB - 1) * FB_COLS  # 10928
N_FC = F_PER_CORE // 128          # 16 output frame chunks (128 frames each)

_PROGRAM = None


def _split_multi_waits(nc, mybir, max_waits=1):
    """This walrus build encodes at most one sem wait per instruction; move
    extra waits onto injected same-engine NoOps right before the instruction."""
    ctr = 0
    for f in nc.m.functions:
        for blk in f.blocks:
            il = list(blk.instructions)
            new = []
            changed = False
            for inst in il:
                si = getattr(inst, "sync_info", None)
                if si is not None and len(si.on_wait) > max_waits:
                    waits = list(si.on_wait)
                    for w in waits[:-max_waits]:
                        nop = mybir.InstNoOp(name=f"I-waitfix-{ctr}", ins=[], outs=[])
                        ctr += 1
                        nop.engine = inst.engine
                        nop.sync_info = mybir.SyncInfo(on_wait=[w], on_update=[])
                        new.append(nop)
                    inst.sync_info = mybir.SyncInfo(
                        on_wait=waits[-max_waits:], on_update=list(si.on_update))
                    changed = True
                new.append(inst)
            if changed:
                blk.instructions = new


def _build_program():
    import concourse.bass as bass
    import concourse.tile as tile
    from concourse import mybir

    nc = bass.Bass("TRN2", target_bir_lowering=False, debug=False)

    ext = nc.dram_tensor("ext", [128, EXT_COLS], mybir.dt.bfloat16,
                         kind="ExternalInput").ap()
    # out[p, fc*84+j] = |cqt|^2 at frame fc*128+p, bin j
    out = nc.dram_tensor("out", [128, N_FC * N_BINS], mybir.dt.float32,
                         kind="ExternalOutput").ap()

    with tile.TileContext(nc) as tc:
        with (
            tc.tile_pool(name="const", bufs=1) as const,
            tc.tile_pool(name="psum", bufs=4, space="PSUM") as psum,
            tc.tile_pool(name="tmp", bufs=4) as tmp,
            tc.tile_pool(name="outp", bufs=1) as outp,
        ):
            xt = const.tile([128, EXT_COLS], mybir.dt.bfloat16)
            # chunked input: [AB_r | fb0 plane r] per r, then fb1..fb3
            for r in range(4):
                lo = r * CH_COLS
                nc.sync.dma_start(xt[:, lo:lo + CH_COLS],
                                  ext[:, lo:lo + CH_COLS])
            for fb in range(1, N_FB):
                lo = FB1_LO + (fb - 1) * FB_COLS
                nc.sync.dma_start(xt[:, lo:lo + FB_COLS],
                                  ext[:, lo:lo + FB_COLS])

            # PE preheat: junk matmuls keep the PE busy while the first input
            # chunks land, so HAM is at full clock when real matmuls start
            junk = const.tile([128, 512], mybir.dt.bfloat16, tag="junk")
            nc.vector.memset(junk[:], 0.0)
            for _ in range(10):
                ps_w = psum.tile([128, 512], mybir.dt.float32, tag="ps")
                nc.tensor.matmul(ps_w[:], junk[:, :128], junk[:],
                                 start=True, stop=True, skip_group_check=True)

            o = outp.tile([128, N_FC, N_BINS], mybir.dt.float32)
            for fc in range(N_FC):
                fb, fi = divmod(fc, 4)  # frame block, 128-frame chunk within
                ps = psum.tile([128, 2 * N_BINS], mybir.dt.float32, tag="ps")
                for i, (r_, a_) in enumerate(
                        (r_, a_) for r_ in range(4) for a_ in range(4)):
                    if fb == 0:
                        lo = r_ * CH_COLS + AB_R_COLS + fi * 128 + a_
                    else:
                        lo = (FB1_LO + (fb - 1) * FB_COLS + r_ * PLANE_COLS
                              + fi * 128 + a_)
                    lhs = xt[:, lo:lo + 128]          # x frames as weights
                    rhs = xt[:, r_ * CH_COLS + a_ * 2 * N_BINS:
                             r_ * CH_COLS + (a_ + 1) * 2 * N_BINS]
                    nc.tensor.matmul(ps[:], lhs, rhs,
                                     start=(i == 0), stop=(i == N_KC - 1))
                # a^2 + b^2: squares on ScalarE (parallel to DVE), add on DVE
                sq = tmp.tile([128, 2 * N_BINS], mybir.dt.float32, tag="sq")
                nc.scalar.square(sq[:, :N_BINS], ps[:, :N_BINS])
                nc.scalar.square(sq[:, N_BINS:], ps[:, N_BINS:])
                nc.vector.tensor_add(o[:, fc, :N_BINS],
                                     sq[:, :N_BINS], sq[:, N_BINS:])
                if fc % 4 == 3:
                    g = fc - 3
                    nc.sync.dma_start(
                        out[:, g * N_BINS:(fc + 1) * N_BINS],
                        o[:, g:fc + 1, :].rearrange("p a b -> p (a b)"))

    _split_multi_waits(nc, mybir)
    return nc


def _get_program():
    global _PROGRAM
    if _PROGRAM is None:
        _PROGRAM = _build_program()
    return _PROGRAM


def _host_prep(x, wcos, wsin, kr, ki):
    """Fold the CQT kernels; shard, cast, and lay out the waveform."""
    kr64 = np.asarray(kr, dtype=np.float64)
    ki64 = np.asarray(ki, dtype=np.float64)
    wc64 = np.asarray(wcos, dtype=np.float64)
    ws64 = np.asarray(wsin, dtype=np.float64)
    a = kr64 @ wc64 - ki64 @ ws64            # [84, 2048]
    b = kr64 @ ws64 + ki64 @ wc64            # [84, 2048]
    abt = np.concatenate([a, b], axis=0).T   # [2048, 168]
    # abkc[kc][p, j] = abt[kc*128+p, j]
    abkc = abt.reshape(N_KC, 128, 2 * N_BINS).astype(ml_dtypes.bfloat16)

    x = np.asarray(x, dtype=np.float32)
    x_pad = np.zeros((N_CORES - 1) * CORE_STRIDE + SHARD_LEN, dtype=np.float32)
    x_pad[:T_SAMPLES] = x
    x_bf = x_pad.astype(ml_dtypes.bfloat16)
    exts = []
    for c in range(N_CORES):
        shard = x_bf[c * CORE_STRIDE: c * CORE_STRIDE + SHARD_LEN]
        # zz[j, r, p] = x[(4j+r)*128 + p]
        zz = shard.reshape(X_COLS_TOTAL // 4, 4, 128)
        ext = np.empty((128, EXT_COLS), dtype=ml_dtypes.bfloat16)
        for r in range(4):
            lo = r * CH_COLS
            for a_ in range(4):
                ext[:, lo + a_ * 2 * N_BINS: lo + (a_ + 1) * 2 * N_BINS] = (
                    abkc[4 * a_ + r])
            ext[:, lo + AB_R_COLS: lo + CH_COLS] = zz[:PLANE_COLS, r, :].T
        for fb in range(1, N_FB):
            lo = FB1_LO + (fb - 1) * FB_COLS
            for r in range(4):
                ext[:, lo + r * PLANE_COLS: lo + (r + 1) * PLANE_COLS] = (
                    zz[fb * 512: fb * 512 + PLANE_COLS, r, :].T)
        exts.append(ext)
    return exts


_LAST_RESULTS = None  # BassKernelResults of the most recent run (for profiling)


def kernel(x, wcos, wsin, kr, ki):
    global _LAST_RESULTS
    from concourse.bass_utils import run_bass_kernel_spmd

    exts = _host_prep(x, wcos, wsin, kr, ki)
    nc = _get_program()
    in_maps = [{"ext": exts[c]} for c in range(N_CORES)]
    res = run_bass_kernel_spmd(nc, in_maps, core_ids=list(range(N_CORES)))
    _LAST_RESULTS = res
    # per core: out[p, fc*84+j] -> [84, 2048 frames] with frame = fc*128+p
    parts = []
    for c in range(N_CORES):
        oc = res.results[c]["out"].reshape(128, N_FC, N_BINS)
        parts.append(oc.transpose(2, 1, 0).reshape(N_BINS, F_PER_CORE))
    full = np.concatenate(parts, axis=1)
    return np.sqrt(full[None, :, :N_FRAMES]).astype(np.float32)


# revision 30
# speedup vs baseline: 2.3233x; 1.0233x over previous
"""Trainium2 Bass kernel: CQT (constant-Q transform) of 2^23 audio samples.

Reference math (jax):
    frames[f, n] = x[f*HOP + n]                  HOP=512, fftLen=2048
    four_r = frames @ wcos.T ; four_i = frames @ wsin.T
    cqt_r  = kr @ four_r - ki @ four_i
    cqt_i  = kr @ four_i + ki @ four_r
    out    = sqrt(cqt_r**2 + cqt_i**2)           # [1, 84, n_frames]

Folded on the host (exact algebra, tiny matrices):
    A = kr@wcos - ki@wsin,  B = kr@wsin + ki@wcos      (each [84, 2048])
    out = sqrt((A @ frames.T)**2 + (B @ frames.T)**2)

Device strategy (8-way shard along the frame axis; kernels replicated):
  - 2048 frames per core.  The bf16 x-shard is laid out host-side so that
    the matmul's moving operand is always a CONTIGUOUS column range: with
    xt[p, c] = x[c*128 + p], contraction chunk kc = 4a + r of frame f needs
    column 4*(f+a) + r, so columns are stored deinterleaved by (frame-block,
    r-plane).  A.T/B.T chunks ride the same DRAM tensor.  (A strided rhs AP
    halves the PE's bf16 stream rate - measured 452 -> 216 ns per matmul.)
  - input DMA is split per frame-block so fb0's matmuls start ~3us in;
    4 fb x 16 kc x {A,B} matmuls accumulate into 8 PSUM banks; a^2+b^2 on
    VectorE; one SWDGE DMA out.  sqrt on the host (monotone, exact).
  - a post-pass splits multi-wait instructions: this walrus build encodes
    at most ONE semaphore wait per instruction.
"""

import sys

if "/opt/trn_rl_repo" not in sys.path:
    sys.path.insert(0, "/opt/trn_rl_repo")

import numpy as np
import ml_dtypes

HOP = 512
FFTLEN = 2048
N_BINS = 84
T_SAMPLES = 8388608
N_FRAMES = (T_SAMPLES - FFTLEN) // HOP + 1  # 16381
N_CORES = 8
F_PER_CORE = 2048                 # frames computed per core (3 junk at the end)
X_COLS_TOTAL = 8204               # sample columns actually needed per core
SHARD_LEN = X_COLS_TOTAL * 128    # 1050112 samples per core
CORE_STRIDE = F_PER_CORE * HOP    # 1048576 samples between shard starts
N_KC = FFTLEN // 128              # 16 contraction chunks
N_FB = F_PER_CORE // 512          # 4 frame blocks of 512 frames
PLANE_COLS = 515                  # columns per r-plane per frame block
FB_COLS = 4 * PLANE_COLS          # 2060
AB_R_COLS = 4 * 2 * N_BINS        # 672: the 4 kc-chunks of A.T/B.T for one r
CH_COLS = AB_R_COLS + PLANE_COLS  # 1187: one [AB_r | fb0 plane r] chunk
FB1_LO = 4 * CH_COLS              # 4748: start of the fb1..fb3 blocks
AB_COLS = N_KC * 2 * N_BINS       # 2688 columns holding A.T/B.T chunks
EXT_COLS = FB1_LO + (N_FB - 1) * FB_COLS  # 10928
N_FC = F_PER_CORE // 128          # 16 output frame chunks (128 frames each)

_PROGRAM = None


def _split_multi_waits(nc, mybir, max_waits=1):
    """This walrus build encodes at most one sem wait per instruction; move
    extra waits onto injected same-engine NoOps right before the instruction."""
    ctr = 0
    for f in nc.m.functions:
        for blk in f.blocks:
            il = list(blk.instructions)
            new = []
            changed = False
            for inst in il:
                si = getattr(inst, "sync_info", None)
                if si is not None and len(si.on_wait) > max_waits:
                    waits = list(si.on_wait)
                    for w in waits[:-max_waits]:
                        nop = mybir.InstNoOp(name=f"I-waitfix-{ctr}", ins=[], outs=[])
                        ctr += 1
                        nop.engine = inst.engine
                        nop.sync_info = mybir.SyncInfo(on_wait=[w], on_update=[])
                        new.append(nop)
                    inst.sync_info = mybir.SyncInfo(
                        on_wait=waits[-max_waits:], on_update=list(si.on_update))
                    changed = True
                new.append(inst)
            if changed:
                blk.instructions = new


def _build_program():
    import concourse.bass as bass
    import concourse.tile as tile
    from concourse import mybir
    from concourse.vector_clock import ScopedClock

    def _lean_drain(self, tick_clock, wait_clock):
        # Tail for a single-shot NEFF: the SP drain already waits on every
        # proc's final tick (incl. output-DMA completion).  The stock
        # drain+barrier+sem-reset+barrier tail costs ~7us and only matters
        # for re-executing a loaded NEFF with dirty semaphores.
        drain_inst = self.nc.sync.drain()
        wait_clock.add_sem_waits(
            drain_inst.ins, ScopedClock({None: tick_clock.global_clock}))
        popped = self.nc._tile_sem_poison_stack.pop()
        assert popped is self._sem_poison

    tile.TileContext._drain_and_barrier = _lean_drain

    nc = bass.Bass("TRN2", target_bir_lowering=False, debug=False)

    ext = nc.dram_tensor("ext", [128, EXT_COLS], mybir.dt.bfloat16,
                         kind="ExternalInput").ap()
    # out[p, fc*84+j] = |cqt|^2 at frame fc*128+p, bin j
    out = nc.dram_tensor("out", [128, N_FC * N_BINS], mybir.dt.float32,
                         kind="ExternalOutput").ap()

    with tile.TileContext(nc) as tc:
        with (
            tc.tile_pool(name="const", bufs=1) as const,
            tc.tile_pool(name="psum", bufs=4, space="PSUM") as psum,
            tc.tile_pool(name="tmp", bufs=4) as tmp,
            tc.tile_pool(name="outp", bufs=1) as outp,
        ):
            xt = const.tile([128, EXT_COLS], mybir.dt.bfloat16)
            # chunked input: [AB_r | fb0 plane r] per r, then fb1..fb3 in
            # half-blocks (smaller chunks keep delivery ahead of consumption)
            for r in range(4):
                lo = r * CH_COLS
                nc.sync.dma_start(xt[:, lo:lo + CH_COLS],
                                  ext[:, lo:lo + CH_COLS])
            half = FB_COLS // 2
            for fb in range(1, N_FB):
                lo = FB1_LO + (fb - 1) * FB_COLS
                nc.sync.dma_start(xt[:, lo:lo + half], ext[:, lo:lo + half])
                nc.sync.dma_start(xt[:, lo + half:lo + FB_COLS],
                                  ext[:, lo + half:lo + FB_COLS])

            # trigger the ScalarE activation-table load in the preamble, not
            # in front of the input DMAs
            sq0 = tmp.tile([128, 1], mybir.dt.float32, tag="sq0")
            junk = const.tile([128, 512], mybir.dt.bfloat16, tag="junk")
            nc.gpsimd.memset(junk[:], 0.0)
            nc.scalar.square(sq0[:], junk[:, 0:1])

            # PE preheat: junk matmuls keep the PE busy while the first input
            # chunks land, so HAM is at full clock when real matmuls start
            for _ in range(11):
                ps_w = psum.tile([128, 512], mybir.dt.float32, tag="ps")
                nc.tensor.matmul(ps_w[:], junk[:, :128], junk[:],
                                 start=True, stop=True, skip_group_check=True)

            o = outp.tile([128, N_FC, N_BINS], mybir.dt.float32)
            for fc in range(N_FC):
                fb, fi = divmod(fc, 4)  # frame block, 128-frame chunk within
                ps = psum.tile([128, 2 * N_BINS], mybir.dt.float32, tag="ps")
                for i, (r_, a_) in enumerate(
                        (r_, a_) for r_ in range(4) for a_ in range(4)):
                    if fb == 0:
                        lo = r_ * CH_COLS + AB_R_COLS + fi * 128 + a_
                    else:
                        lo = (FB1_LO + (fb - 1) * FB_COLS + r_ * PLANE_COLS
                              + fi * 128 + a_)
                    lhs = xt[:, lo:lo + 128]          # x frames as weights
                    rhs = xt[:, r_ * CH_COLS + a_ * 2 * N_BINS:
                             r_ * CH_COLS + (a_ + 1) * 2 * N_BINS]
                    nc.tensor.matmul(ps[:], lhs, rhs,
                                     start=(i == 0), stop=(i == N_KC - 1))
                # a^2 + b^2: squares on ScalarE (parallel to DVE), add on DVE
                sq = tmp.tile([128, 2 * N_BINS], mybir.dt.float32, tag="sq")
                nc.scalar.square(sq[:, :N_BINS], ps[:, :N_BINS])
                nc.scalar.square(sq[:, N_BINS:], ps[:, N_BINS:])
                nc.vector.tensor_add(o[:, fc, :N_BINS],
                                     sq[:, :N_BINS], sq[:, N_BINS:])
                if fc % 4 == 3:
                    g = fc - 3
                    nc.sync.dma_start(
                        out[:, g * N_BINS:(fc + 1) * N_BINS],
                        o[:, g:fc + 1, :].rearrange("p a b -> p (a b)"))

    _split_multi_waits(nc, mybir)
    return nc


def _get_program():
    global _PROGRAM
    if _PROGRAM is None:
        _PROGRAM = _build_program()
    return _PROGRAM


def _host_prep(x, wcos, wsin, kr, ki):
    """Fold the CQT kernels; shard, cast, and lay out the waveform."""
    kr64 = np.asarray(kr, dtype=np.float64)
    ki64 = np.asarray(ki, dtype=np.float64)
    wc64 = np.asarray(wcos, dtype=np.float64)
    ws64 = np.asarray(wsin, dtype=np.float64)
    a = kr64 @ wc64 - ki64 @ ws64            # [84, 2048]
    b = kr64 @ ws64 + ki64 @ wc64            # [84, 2048]
    abt = np.concatenate([a, b], axis=0).T   # [2048, 168]
    # abkc[kc][p, j] = abt[kc*128+p, j]
    abkc = abt.reshape(N_KC, 128, 2 * N_BINS).astype(ml_dtypes.bfloat16)

    x = np.asarray(x, dtype=np.float32)
    x_pad = np.zeros((N_CORES - 1) * CORE_STRIDE + SHARD_LEN, dtype=np.float32)
    x_pad[:T_SAMPLES] = x
    x_bf = x_pad.astype(ml_dtypes.bfloat16)
    exts = []
    for c in range(N_CORES):
        shard = x_bf[c * CORE_STRIDE: c * CORE_STRIDE + SHARD_LEN]
        # zz[j, r, p] = x[(4j+r)*128 + p]
        zz = shard.reshape(X_COLS_TOTAL // 4, 4, 128)
        ext = np.empty((128, EXT_COLS), dtype=ml_dtypes.bfloat16)
        for r in range(4):
            lo = r * CH_COLS
            for a_ in range(4):
                ext[:, lo + a_ * 2 * N_BINS: lo + (a_ + 1) * 2 * N_BINS] = (
                    abkc[4 * a_ + r])
            ext[:, lo + AB_R_COLS: lo + CH_COLS] = zz[:PLANE_COLS, r, :].T
        for fb in range(1, N_FB):
            lo = FB1_LO + (fb - 1) * FB_COLS
            for r in range(4):
                ext[:, lo + r * PLANE_COLS: lo + (r + 1) * PLANE_COLS] = (
                    zz[fb * 512: fb * 512 + PLANE_COLS, r, :].T)
        exts.append(ext)
    return exts


_LAST_RESULTS = None  # BassKernelResults of the most recent run (for profiling)


def kernel(x, wcos, wsin, kr, ki):
    global _LAST_RESULTS
    from concourse.bass_utils import run_bass_kernel_spmd

    exts = _host_prep(x, wcos, wsin, kr, ki)
    nc = _get_program()
    in_maps = [{"ext": exts[c]} for c in range(N_CORES)]
    res = run_bass_kernel_spmd(nc, in_maps, core_ids=list(range(N_CORES)))
    _LAST_RESULTS = res
    # per core: out[p, fc*84+j] -> [84, 2048 frames] with frame = fc*128+p
    parts = []
    for c in range(N_CORES):
        oc = res.results[c]["out"].reshape(128, N_FC, N_BINS)
        parts.append(oc.transpose(2, 1, 0).reshape(N_BINS, F_PER_CORE))
    full = np.concatenate(parts, axis=1)
    return np.sqrt(full[None, :, :N_FRAMES]).astype(np.float32)


# revision 31
# speedup vs baseline: 2.3581x; 1.0150x over previous
"""Trainium2 Bass kernel: CQT (constant-Q transform) of 2^23 audio samples.

Reference math (jax):
    frames[f, n] = x[f*HOP + n]                  HOP=512, fftLen=2048
    four_r = frames @ wcos.T ; four_i = frames @ wsin.T
    cqt_r  = kr @ four_r - ki @ four_i
    cqt_i  = kr @ four_i + ki @ four_r
    out    = sqrt(cqt_r**2 + cqt_i**2)           # [1, 84, n_frames]

Folded on the host (exact algebra, tiny matrices):
    A = kr@wcos - ki@wsin,  B = kr@wsin + ki@wcos      (each [84, 2048])
    out = sqrt((A @ frames.T)**2 + (B @ frames.T)**2)

Device strategy (8-way shard along the frame axis; kernels replicated):
  - 2048 frames per core.  The bf16 x-shard is laid out host-side so that
    the matmul's moving operand is always a CONTIGUOUS column range: with
    xt[p, c] = x[c*128 + p], contraction chunk kc = 4a + r of frame f needs
    column 4*(f+a) + r, so columns are stored deinterleaved by (frame-block,
    r-plane).  A.T/B.T chunks ride the same DRAM tensor.  (A strided rhs AP
    halves the PE's bf16 stream rate - measured 452 -> 216 ns per matmul.)
  - input DMA is split per frame-block so fb0's matmuls start ~3us in;
    4 fb x 16 kc x {A,B} matmuls accumulate into 8 PSUM banks; a^2+b^2 on
    VectorE; one SWDGE DMA out.  sqrt on the host (monotone, exact).
  - a post-pass splits multi-wait instructions: this walrus build encodes
    at most ONE semaphore wait per instruction.
"""

import sys

if "/opt/trn_rl_repo" not in sys.path:
    sys.path.insert(0, "/opt/trn_rl_repo")

import numpy as np
import ml_dtypes

HOP = 512
FFTLEN = 2048
N_BINS = 84
T_SAMPLES = 8388608
N_FRAMES = (T_SAMPLES - FFTLEN) // HOP + 1  # 16381
N_CORES = 8
F_PER_CORE = 2048                 # frames computed per core (3 junk at the end)
X_COLS_TOTAL = 8204               # sample columns actually needed per core
SHARD_LEN = X_COLS_TOTAL * 128    # 1050112 samples per core
CORE_STRIDE = F_PER_CORE * HOP    # 1048576 samples between shard starts
N_KC = FFTLEN // 128              # 16 contraction chunks
N_FB = F_PER_CORE // 512          # 4 frame blocks of 512 frames
PLANE_COLS = 515                  # columns per r-plane per frame block
FB_COLS = 4 * PLANE_COLS          # 2060
AB_R_COLS = 4 * 2 * N_BINS        # 672: the 4 kc-chunks of A.T/B.T for one r
CH_COLS = AB_R_COLS + PLANE_COLS  # 1187: one [AB_r | fb0 plane r] chunk
FB1_LO = 4 * CH_COLS              # 4748: start of the fb1..fb3 blocks
AB_COLS = N_KC * 2 * N_BINS       # 2688 columns holding A.T/B.T chunks
EXT_COLS = FB1_LO + (N_FB - 1) * FB_COLS  # 10928
N_FC = F_PER_CORE // 128          # 16 output frame chunks (128 frames each)

_PROGRAM = None


def _split_multi_waits(nc, mybir, max_waits=1):
    """This walrus build encodes at most one sem wait per instruction; move
    extra waits onto injected same-engine NoOps right before the instruction."""
    ctr = 0
    for f in nc.m.functions:
        for blk in f.blocks:
            il = list(blk.instructions)
            new = []
            changed = False
            for inst in il:
                si = getattr(inst, "sync_info", None)
                if si is not None and len(si.on_wait) > max_waits:
                    waits = list(si.on_wait)
                    for w in waits[:-max_waits]:
                        nop = mybir.InstNoOp(name=f"I-waitfix-{ctr}", ins=[], outs=[])
                        ctr += 1
                        nop.engine = inst.engine
                        nop.sync_info = mybir.SyncInfo(on_wait=[w], on_update=[])
                        new.append(nop)
                    inst.sync_info = mybir.SyncInfo(
                        on_wait=waits[-max_waits:], on_update=list(si.on_update))
                    changed = True
                new.append(inst)
            if changed:
                blk.instructions = new


def _build_program():
    import concourse.bass as bass
    import concourse.tile as tile
    from concourse import mybir
    from concourse.vector_clock import ScopedClock

    def _lean_drain(self, tick_clock, wait_clock):
        # Tail for a single-shot NEFF: the SP drain already waits on every
        # proc's final tick (incl. output-DMA completion).  The stock
        # drain+barrier+sem-reset+barrier tail costs ~7us and only matters
        # for re-executing a loaded NEFF with dirty semaphores.
        drain_inst = self.nc.sync.drain()
        wait_clock.add_sem_waits(
            drain_inst.ins, ScopedClock({None: tick_clock.global_clock}))
        popped = self.nc._tile_sem_poison_stack.pop()
        assert popped is self._sem_poison

    tile.TileContext._drain_and_barrier = _lean_drain

    nc = bass.Bass("TRN2", target_bir_lowering=False, debug=False)

    ext = nc.dram_tensor("ext", [128, EXT_COLS], mybir.dt.bfloat16,
                         kind="ExternalInput").ap()
    # out[p, fc*84+j] = |cqt|^2 at frame fc*128+p, bin j
    out = nc.dram_tensor("out", [128, N_FC * N_BINS], mybir.dt.float32,
                         kind="ExternalOutput").ap()

    with tile.TileContext(nc) as tc:
        with (
            tc.tile_pool(name="const", bufs=1) as const,
            tc.tile_pool(name="psum", bufs=4, space="PSUM") as psum,
            tc.tile_pool(name="tmp", bufs=4) as tmp,
            tc.tile_pool(name="outp", bufs=1) as outp,
        ):
            xt = const.tile([128, EXT_COLS], mybir.dt.bfloat16)
            # chunked input on both HWDGE rings (SP + ACT issue in parallel):
            # [AB_r | fb0 plane r] per r, then fb1..fb3 in half-blocks
            engs = [nc.sync, nc.scalar]
            for r in range(4):
                lo = r * CH_COLS
                engs[r % 2].dma_start(xt[:, lo:lo + CH_COLS],
                                      ext[:, lo:lo + CH_COLS])
            half = FB_COLS // 2
            for fb in range(1, N_FB):
                lo = FB1_LO + (fb - 1) * FB_COLS
                engs[fb % 2].dma_start(xt[:, lo:lo + half],
                                       ext[:, lo:lo + half])
                engs[(fb + 1) % 2].dma_start(xt[:, lo + half:lo + FB_COLS],
                                             ext[:, lo + half:lo + FB_COLS])

            # PE preheat: junk matmuls on raw (uninitialized, untracked) SBUF
            # keep the PE busy from the first post-preamble cycle, so HAM is
            # at full clock when the real matmuls start
            junk = nc.alloc_sbuf_tensor("junk", [128, 512],
                                        mybir.dt.bfloat16).ap()
            for _ in range(11):
                ps_w = psum.tile([128, 512], mybir.dt.float32, tag="ps")
                nc.tensor.matmul(ps_w[:], junk[:, :128], junk[:],
                                 start=True, stop=True, skip_group_check=True)

            o = outp.tile([128, N_FC, N_BINS], mybir.dt.float32)
            for fc in range(N_FC):
                fb, fi = divmod(fc, 4)  # frame block, 128-frame chunk within
                ps = psum.tile([128, 2 * N_BINS], mybir.dt.float32, tag="ps")
                for i, (r_, a_) in enumerate(
                        (r_, a_) for r_ in range(4) for a_ in range(4)):
                    if fb == 0:
                        lo = r_ * CH_COLS + AB_R_COLS + fi * 128 + a_
                    else:
                        lo = (FB1_LO + (fb - 1) * FB_COLS + r_ * PLANE_COLS
                              + fi * 128 + a_)
                    lhs = xt[:, lo:lo + 128]          # x frames as weights
                    rhs = xt[:, r_ * CH_COLS + a_ * 2 * N_BINS:
                             r_ * CH_COLS + (a_ + 1) * 2 * N_BINS]
                    nc.tensor.matmul(ps[:], lhs, rhs,
                                     start=(i == 0), stop=(i == N_KC - 1))
                # a^2 + b^2: squares on ScalarE (parallel to DVE), add on DVE
                sq = tmp.tile([128, 2 * N_BINS], mybir.dt.float32, tag="sq")
                nc.scalar.square(sq[:, :N_BINS], ps[:, :N_BINS])
                nc.scalar.square(sq[:, N_BINS:], ps[:, N_BINS:])
                nc.vector.tensor_add(o[:, fc, :N_BINS],
                                     sq[:, :N_BINS], sq[:, N_BINS:])
                if fc % 4 == 3:
                    g = fc - 3
                    nc.sync.dma_start(
                        out[:, g * N_BINS:(fc + 1) * N_BINS],
                        o[:, g:fc + 1, :].rearrange("p a b -> p (a b)"))

    _split_multi_waits(nc, mybir)
    return nc


def _get_program():
    global _PROGRAM
    if _PROGRAM is None:
        _PROGRAM = _build_program()
    return _PROGRAM


def _host_prep(x, wcos, wsin, kr, ki):
    """Fold the CQT kernels; shard, cast, and lay out the waveform."""
    kr64 = np.asarray(kr, dtype=np.float64)
    ki64 = np.asarray(ki, dtype=np.float64)
    wc64 = np.asarray(wcos, dtype=np.float64)
    ws64 = np.asarray(wsin, dtype=np.float64)
    a = kr64 @ wc64 - ki64 @ ws64            # [84, 2048]
    b = kr64 @ ws64 + ki64 @ wc64            # [84, 2048]
    abt = np.concatenate([a, b], axis=0).T   # [2048, 168]
    # abkc[kc][p, j] = abt[kc*128+p, j]
    abkc = abt.reshape(N_KC, 128, 2 * N_BINS).astype(ml_dtypes.bfloat16)

    x = np.asarray(x, dtype=np.float32)
    x_pad = np.zeros((N_CORES - 1) * CORE_STRIDE + SHARD_LEN, dtype=np.float32)
    x_pad[:T_SAMPLES] = x
    x_bf = x_pad.astype(ml_dtypes.bfloat16)
    exts = []
    for c in range(N_CORES):
        shard = x_bf[c * CORE_STRIDE: c * CORE_STRIDE + SHARD_LEN]
        # zz[j, r, p] = x[(4j+r)*128 + p]
        zz = shard.reshape(X_COLS_TOTAL // 4, 4, 128)
        ext = np.empty((128, EXT_COLS), dtype=ml_dtypes.bfloat16)
        for r in range(4):
            lo = r * CH_COLS
            for a_ in range(4):
                ext[:, lo + a_ * 2 * N_BINS: lo + (a_ + 1) * 2 * N_BINS] = (
                    abkc[4 * a_ + r])
            ext[:, lo + AB_R_COLS: lo + CH_COLS] = zz[:PLANE_COLS, r, :].T
        for fb in range(1, N_FB):
            lo = FB1_LO + (fb - 1) * FB_COLS
            for r in range(4):
                ext[:, lo + r * PLANE_COLS: lo + (r + 1) * PLANE_COLS] = (
                    zz[fb * 512: fb * 512 + PLANE_COLS, r, :].T)
        exts.append(ext)
    return exts


_LAST_RESULTS = None  # BassKernelResults of the most recent run (for profiling)


def kernel(x, wcos, wsin, kr, ki):
    global _LAST_RESULTS
    from concourse.bass_utils import run_bass_kernel_spmd

    exts = _host_prep(x, wcos, wsin, kr, ki)
    nc = _get_program()
    in_maps = [{"ext": exts[c]} for c in range(N_CORES)]
    res = run_bass_kernel_spmd(nc, in_maps, core_ids=list(range(N_CORES)))
    _LAST_RESULTS = res
    # per core: out[p, fc*84+j] -> [84, 2048 frames] with frame = fc*128+p
    parts = []
    for c in range(N_CORES):
        oc = res.results[c]["out"].reshape(128, N_FC, N_BINS)
        parts.append(oc.transpose(2, 1, 0).reshape(N_BINS, F_PER_CORE))
    full = np.concatenate(parts, axis=1)
    return np.sqrt(full[None, :, :N_FRAMES]).astype(np.float32)
